# revision 58
# baseline (speedup 1.0000x reference)
"""Multi-head attention (B=2, S=2048, D=1024, H=16) on 8 TRN2 NeuronCores.

Sharding: core c handles batch b = c//4 and heads [4*(c%4), 4*(c%4)+4) —
tensor-parallel over heads x data-parallel over batch.  Each core computes a
partial output projection (its heads' contribution); the host sums the 4
partials per batch and adds b_out.

On-device layout (per core; all matmul operands bf16, fp32 PSUM math):
  - qk projection computed transposed: qkT [512, S] bf16 with row chunks
    [q_h0|q_h1, k_h0|k_h1, q_h2|q_h3, k_h2|k_h3].  Phase 1 computes only the
    first head-pair's q,k (m=0,1), kc-outer across all 8 PSUM banks so
    matmuls start as each yT DMA chunk lands; q2/k2/q3/k3 and the v
    projection drain as PE filler inside the first attention blocks.
  - scores computed transposed: expT[sk, sq] = exp(0.125 * kT.T @ qT).
    Exps for 6 of 8 score chunk-pairs run on the ACT engine (table exp);
    the other 2 run on the DVE as a Schraudolph fast-exp (bf16 bits =
    score*log2e*128 + 16250.5 written through an int16 cast) — linear in
    the score so the ~+-12 score tails cannot overflow, and the ~3% PWL
    error largely cancels in the softmax ratio.  bf16 (not fp8) because
    exp reaches ~2e5 in the tails, far beyond fp8e4m3's 448 max.
  - attn@v: valuesT_unnorm [65, sq] = v_aug.T @ expT accumulated over sk
    chunks in reversed order (only the first matmul waits on the exps);
    the ones-column of v_aug makes PSUM row 64 the softmax denominator.
    Each chain is split in half and issued across 4 score-slots mid-block
    so the next block's exps never wait on an ex-buffer WAR hazard and the
    ACT engine never starves; the last block runs forward-order in-block.
  - normalization: the denominator row bounces through DRAM to be re-read
    as [128, 4] so the DVE reciprocal runs on 128 lanes (a [1,512]
    reciprocal is 8 cycles/elem on one lane = ~4.3us), bounced back, and
    broadcast across 64 partitions via a stride-0 DMA; DVE mul -> vT bf16.
  - output projection per sq block as soon as both pairs' vT slices land,
    deferred one extra block so its inputs' DMA chain is certain to have
    landed (the PE is strictly in-order; a not-ready filler blocks it);
    PSUM->SBUF copies alternate between DVE and ACT.
  - scheduling: the PE is the bottleneck engine (~190us busy; ~380ns
    per 512-col matmul incl. exposed issue latency vs 131ns production
    roofline).  All non-score PE work lives in a filler queue drained two
    groups per score slot so score tiles stay ahead of the exp engines.
"""
import sys

sys.path.insert(0, "/opt/trn_rl_repo")

import numpy as np

B, S, D = 2, 2048, 1024
H, Hd = 16, 64
P = 128
NKC = D // P      # 8 contraction chunks for the projections
NSC = S // P      # 16 sequence chunks of 128
SQB = 512         # sq block size
NSQB = S // SQB   # 4

_CACHE = {}


def _build_nc():
    import concourse.mybir as mybir
    import concourse.tile as tile
    from concourse import bacc

    f32 = mybir.dt.float32
    bf16 = mybir.dt.bfloat16
    fp8 = mybir.dt.float8e4
    AF = mybir.ActivationFunctionType
    DR = mybir.MatmulPerfMode.DoubleRow

    nc = bacc.Bacc(None, target_bir_lowering=False, debug=False)

    yT_d = nc.dram_tensor("yT", [D, S], bf16, kind="ExternalInput")[:]
    Wqk_d = nc.dram_tensor("WqkT", [D, 512], bf16, kind="ExternalInput")[:]
    bqk_d = nc.dram_tensor("bqk", [P, 4], f32, kind="ExternalInput")[:]
    Wv_d = nc.dram_tensor("WvT", [D, 256], bf16, kind="ExternalInput")[:]
    Wout_d = nc.dram_tensor("WoutT", [256, D], bf16, kind="ExternalInput")[:]
    out_d = nc.dram_tensor("out", [S, D], f32, kind="ExternalOutput")[:]
    import os
    dbg = os.environ.get("KDBG") == "1"
    # Schraudolph fast-exp on the DVE for a subset of score chunks: bf16
    # bit pattern = y*2^7 + (127*2^7 - sigma), y = score*log2(e).  Linear in
    # the score (no overflow even at score ~ +-12); rel err ~ +-3% piecewise
    # linear, which the softmax normalization averages out.  Offloading 2 of
    # 8 chunks per block cuts the ACT exp floor by 25%.
    dve_mjs = {int(x) for x in
               os.environ.get("KDVEMJ", "1,5").split(",") if x != ""}
    c1 = float(os.environ.get("KEXPC1", "16250.5"))
    if dbg:
        vT_dump = nc.dram_tensor("vT_dump", [P, 2, S], f32,
                                 kind="ExternalOutput")[:]
        den_dump = nc.dram_tensor("den_dump", [2, NSQB, 2, SQB], f32,
                                  kind="ExternalOutput")[:]
        rec_dump = nc.dram_tensor("rec_dump", [2, NSQB, 2, SQB], f32,
                                  kind="ExternalOutput")[:]

    with tile.TileContext(nc) as tc:
        with (
            tc.tile_pool(name="const", bufs=1) as const,
            tc.tile_pool(name="persist", bufs=1) as persist,
        ):
            qkT_sb = persist.tile([P, 4, S], bf16)
            # v_aug per sk-chunk/head: cols 0-63 = v, col 64 = 1.0 (the
            # ones column turns the attn@v matmul's row 64 into the softmax
            # denominator).  bf16: scores reach ~12 in the tails so exp
            # goes up to ~2e5 — far beyond fp8 range.
            v_sb = persist.tile([P, NSC, 4, 65], bf16)
            vT_sb = persist.tile([P, 2, S], bf16)
            ones_f32 = const.tile([P, 1], f32)
            nc.any.memset(ones_f32[:], 1.0)
            nc.vector.tensor_copy(
                v_sb[:, :, :, 64:65],
                ones_f32.unsqueeze(1).unsqueeze(1).to_broadcast(
                    (P, NSC, 4, 1)))

            # ---- DMA order matters for the head: yT/Wqk chunks first (the
            # phase-1 proj gates the first exp), Wv next, Wout last. ----
            p1 = ctx_p1 = tc.alloc_tile_pool(name="p1", bufs=1)
            Wqk_sb = p1.tile([P, NKC, 512], bf16)
            yT_sb = p1.tile([P, NKC, S], bf16)
            yTr = yT_d.rearrange("(kc p) s -> p kc s", p=P)
            Wqkr = Wqk_d.rearrange("(kc p) e -> p kc e", p=P)
            for kc in range(NKC):
                nc.sync.dma_start(Wqk_sb[:, kc, :], Wqkr[:, kc, :])
                nc.sync.dma_start(yT_sb[:, kc, :], yTr[:, kc, :])
            bqk_sb = const.tile([P, 4], f32)
            nc.sync.dma_start(bqk_sb[:], bqk_d)
            Wv_sb = p1.tile([P, NKC, 256], bf16)
            nc.sync.dma_start(
                Wv_sb[:], Wv_d.rearrange("(kc p) e -> p kc e", p=P))
            Wout_sb = const.tile([P, 2, D], bf16)
            nc.sync.dma_start(Wout_sb[:],
                              Wout_d.rearrange("(kc p) e -> p kc e", p=P))

            # ---- phase 1: q,k projection for head pair 0 (m=0,1),
            # kc-outer across 8 psum banks so matmuls start as each yT
            # chunk lands; at the last kc round each (m, sb) group's bias
            # copy is emitted immediately, k-then-q per sb, so the first
            # score block unblocks as early as possible. ----
            with tc.tile_pool(name="p1ps", bufs=8, space="PSUM") as p1ps:
                ps_mm = [[p1ps.tile([P, 512], f32, tag="proj", name="ps01")
                          for _ in range(4)] for _ in range(2)]
                for kc in range(NKC - 1):
                    for m in range(2):
                        for sb in range(4):
                            nc.tensor.matmul(
                                ps_mm[m][sb][:],
                                Wqk_sb[:, kc, m * P:(m + 1) * P],
                                yT_sb[:, kc, sb * 512:(sb + 1) * 512],
                                start=(kc == 0), stop=False)
                for sb in range(4):
                    for m in (1, 0):
                        nc.tensor.matmul(
                            ps_mm[m][sb][:],
                            Wqk_sb[:, NKC - 1, m * P:(m + 1) * P],
                            yT_sb[:, NKC - 1, sb * 512:(sb + 1) * 512],
                            start=False, stop=True)
                        nc.vector.tensor_scalar_add(
                            qkT_sb[:, m, sb * 512:(sb + 1) * 512],
                            ps_mm[m][sb][:], bqk_sb[:, m:m + 1])

            # ---- phase 2: attention, ACT-paced; PE filler queue ----
            with (
                tc.tile_pool(name="p2e", bufs=4) as p2e,
                tc.tile_pool(name="p2s", bufs=2) as p2s,
                tc.tile_pool(name="p2ps", bufs=2, space="PSUM") as p2ps,
                tc.tile_pool(name="p2dram", bufs=4, space="DRAM") as p2dram,
                tc.tile_pool(name="p2sh", bufs=2, space="PSUM") as p2sh,
            ):
                filler = []
                pending = []

                def vproj_group(sc):
                    def run():
                        psv = p2sh.tile([P, 512], f32, tag="sh", name="psv")
                        for kc in range(NKC):
                            nc.tensor.matmul(
                                psv[:, 0:256],
                                yT_sb[:, kc, sc * P:(sc + 1) * P],
                                Wv_sb[:, kc, :],
                                start=(kc == 0), stop=(kc == NKC - 1))
                        nc.vector.tensor_copy(
                            v_sb[:, sc, :, 0:64],
                            psv[:, 0:256].rearrange("p (i d) -> p i d", i=4))
                    return run

                def proj_group(m, sb):
                    def run():
                        ps = p2sh.tile([P, 512], f32, tag="sh", name="psqk")
                        for kc in range(NKC):
                            nc.tensor.matmul(
                                ps[:],
                                Wqk_sb[:, kc, m * P:(m + 1) * P],
                                yT_sb[:, kc, sb * 512:(sb + 1) * 512],
                                start=(kc == 0), stop=(kc == NKC - 1))
                        nc.vector.tensor_scalar_add(
                            qkT_sb[:, m, sb * 512:(sb + 1) * 512],
                            ps[:], bqk_sb[:, m:m + 1])
                    return run

                def outproj_group(sc, nb):
                    def run():
                        pso = p2sh.tile([P, 512], f32, tag="sh", name="pso")
                        for kc in range(2):
                            nc.tensor.matmul(
                                pso[:],
                                vT_sb[:, kc, sc * P:(sc + 1) * P],
                                Wout_sb[:, kc, nb * 512:(nb + 1) * 512],
                                start=(kc == 0), stop=(kc == 1))
                        ost = p2s.tile([P, 512], f32, tag="ost",
                                       name="ost", bufs=3)
                        # alternate the PSUM->SBUF copy between DVE and ACT
                        # so neither engine serializes the out-proj stream
                        if (sc + nb) % 2:
                            nc.scalar.copy(ost[:], pso[:])
                        else:
                            nc.vector.tensor_copy(ost[:], pso[:])
                        nc.sync.dma_start(
                            out_d[sc * P:(sc + 1) * P,
                                  nb * 512:(nb + 1) * 512], ost[:])
                    return run

                def drain(k):
                    for _ in range(min(k, len(filler))):
                        filler.pop(0)()

                def normalize_tail(p, sqb, sub, vals, rdram2):
                    sq = slice(sqb * SQB, (sqb + 1) * SQB)
                    rbs = p2s.tile([64, SQB], f32, tag="rbs", name="rbs")
                    nc.sync.dma_start(rbs[:], rdram2.to_broadcast((64, SQB)))
                    if dbg:
                        nc.sync.dma_start(
                            rec_dump[p, sqb, sub].unsqueeze(0),
                            rbs[0:1, :])
                    vtmp = p2s.tile([64, SQB], bf16, tag="vtmp", name="vtmp")
                    nc.vector.tensor_mul(vtmp[:], vals[:], rbs[:])
                    nc.sync.dma_start(
                        vT_sb[sub * 64:(sub + 1) * 64, p, sq], vtmp[:])

                cur_psv2 = {}

                def attn_v_part(p, sqb, ex, sub, part, fwd=False):
                    """Half of one head's attn@v chain.  Normally reversed
                    (part 0 = upper sk chunks w/ group start) so only the
                    first matmul waits on ACT; the last block runs forward
                    (fwd=True) so it can overlap its own exps in-block.
                    Split in half so each PE burst fits inside the
                    score-tile lookahead and the ACT engine never starves.
                    Part 1 finishes the group and launches the
                    transposed-reciprocal normalize chain."""
                    i = 2 * p + sub
                    half = NSC // 2
                    if part == 0:
                        psv2 = p2sh.tile([P, SQB], f32, tag="sh",
                                         name="psv2")
                        cur_psv2[sub] = psv2
                        mks = range(0, half) if fwd else \
                            range(NSC - 1, half - 1, -1)
                        first = 0 if fwd else NSC - 1
                        for mk in mks:
                            nc.tensor.matmul(
                                psv2[0:65, :],
                                v_sb[:, mk, i, :],
                                ex[sub][:, mk, :],
                                start=(mk == first), stop=False)
                        return
                    psv2 = cur_psv2[sub]
                    mks = range(half, NSC) if fwd else \
                        range(half - 1, -1, -1)
                    lastmk = NSC - 1 if fwd else 0
                    for mk in mks:
                        nc.tensor.matmul(
                            psv2[0:65, :],
                            v_sb[:, mk, i, :],
                            ex[sub][:, mk, :],
                            start=False, stop=(mk == lastmk))
                    vals = p2s.tile([64, SQB], f32, tag="vals",
                                    name="vals", bufs=6)
                    nc.vector.tensor_copy(vals[:], psv2[0:64, :])
                    denrow = p2s.tile([P, SQB], f32, tag="den",
                                      name="den", bufs=4)
                    nc.vector.tensor_copy(denrow[64:65, :],
                                          psv2[64:65, :])
                    d1 = p2dram.tile([1, SQB], f32, name="d1")
                    nc.sync.dma_start(d1[:], denrow[64:65, :])
                    if dbg:
                        nc.sync.dma_start(
                            den_dump[p, sqb, sub].unsqueeze(0),
                            denrow[64:65, :])
                    # recip on 128 lanes instead of 1
                    rt = p2s.tile([P, 4], f32, tag="rt", name="rt",
                                  bufs=4)
                    d1t = d1.rearrange("a (p b) -> (a p) b", p=P)
                    nc.sync.dma_start(rt[:], d1t)
                    rt2 = p2s.tile([P, 4], f32, tag="rt2", name="rt2",
                                   bufs=4)
                    nc.vector.reciprocal(rt2[:], rt[:])
                    d2 = p2dram.tile([1, SQB], f32, name="d2")
                    d2t = d2.rearrange("a (p b) -> (a p) b", p=P)
                    nc.sync.dma_start(d2t, rt2[:])
                    pending.append((p, sqb, sub, vals, d2))

                filler_next = []
                filler_next2 = []

                def pop_pending():
                    if not pending:
                        return False
                    pp, psqb, psub, pvals, prd = pending.pop(0)
                    normalize_tail(pp, psqb, psub, pvals, prd)
                    if pp == 1 and psub == 1:
                        # defer out-proj to the next block's drain slots
                        # so its vT inputs' DMA chain has landed and the PE
                        # never blocks in-order on a not-ready filler
                        filler_next.extend(
                            outproj_group(sc, nb)
                            for sc in range(psqb * 4, psqb * 4 + 4)
                            for nb in range(2))
                    return True

                # priority-ordered by need time: v chunks 8-15 gate
                # attn@v(b0) part 0 at block-1 mj0 (all 16 must drain in
                # block 0); m2/m3 gate the p=1 blocks (block 4+)
                filler.extend(vproj_group(sc) for sc in range(NSC - 1, -1, -1))
                filler.extend(proj_group(m, sb)
                              for m in (2, 3) for sb in range(4))

                prev = None
                for p in range(2):
                    for sqb in range(NSQB):
                        last = (p == 1 and sqb == NSQB - 1)
                        sq = slice(sqb * SQB, (sqb + 1) * SQB)
                        exa = p2e.tile([P, NSC, SQB], bf16, tag="exp")
                        exb = p2e.tile([P, NSC, SQB], bf16, tag="exp")
                        ex = (exa, exb)
                        for mj in range(NSC // 2):
                            pss = [
                                p2ps.tile([P, 2, SQB], f32, tag="score",
                                          bufs=3, name="pss")
                                for _ in range(2)]
                            for half in range(2):
                                mk = 2 * mj + half
                                for sub in range(2):
                                    prt = slice(sub * 64, (sub + 1) * 64)
                                    nc.tensor.matmul(
                                        pss[sub][:, half, :],
                                        qkT_sb[prt, 2 * p + 1,
                                               mk * P:(mk + 1) * P],
                                        qkT_sb[prt, 2 * p, sq])
                            for sub in range(2):
                                if mj in dve_mjs:
                                    nc.vector.tensor_scalar(
                                        ex[sub][:, 2 * mj:2 * mj + 2, :]
                                        .bitcast(mybir.dt.int16),
                                        pss[sub][:],
                                        0.125 * 1.4426950408889634 * 128,
                                        c1,
                                        mybir.AluOpType.mult,
                                        mybir.AluOpType.add)
                                else:
                                    nc.scalar.activation(
                                        ex[sub][:, 2 * mj:2 * mj + 2, :],
                                        pss[sub][:], AF.Exp, scale=0.125)
                            # prev block's attn@v in four mid-block bursts
                            # (never at the boundary: next block's exps must
                            # not wait on an ex-buffer WAR hazard); pops
                            # launch deferred normalizes once their DMA
                            # round trips have had time to land
                            if prev is not None and mj < 4:
                                attn_v_part(*prev, sub=mj // 2, part=mj % 2)
                            elif last and mj in (4, 5):
                                # last block: the lower-half attn@v chains
                                # run in-block forward order (their exps,
                                # mj0-3, are already issued; upper halves
                                # must wait for mj6/7's exps and run after
                                # the loop)
                                attn_v_part(p, sqb, ex, sub=mj - 4,
                                            part=0, fwd=True)
                            elif mj in (5, 6):
                                drain(1 if pop_pending() else 2)
                            else:
                                drain(2)
                        prev = (p, sqb, ex)
                        filler.extend(filler_next)
                        del filler_next[:]
                        filler_next.extend(filler_next2)
                        del filler_next2[:]
                attn_v_part(1, NSQB - 1, prev[2], sub=0, part=1, fwd=True)
                attn_v_part(1, NSQB - 1, prev[2], sub=1, part=1, fwd=True)
                while pending:
                    pop_pending()
                filler.extend(filler_next)
                filler.extend(filler_next2)
                del filler_next[:]
                del filler_next2[:]
                drain(len(filler))
                if dbg:
                    vT32 = p2s.tile([P, 2, S], f32, tag="vT32", name="vT32",
                                    bufs=1)
                    nc.vector.tensor_copy(vT32[:], vT_sb[:])
                    nc.sync.dma_start(vT_dump, vT32[:])

            ctx_p1.release()

    nc.compile()
    return nc


def _get_nc():
    if "nc" not in _CACHE:
        _CACHE["nc"] = _build_nc()
    return _CACHE["nc"]


def _host_prep(y, W_qkv, b_qkv, W_out, c):
    b = c // 4
    q = c % 4
    hs = [4 * q + i for i in range(4)]

    def Wrow(h, part):
        return W_qkv[h * 192 + part * 64: h * 192 + (part + 1) * 64]

    def brow(h, part):
        return b_qkv[h * 192 + part * 64: h * 192 + (part + 1) * 64]

    qk_rows = np.concatenate([
        Wrow(hs[0], 0), Wrow(hs[1], 0), Wrow(hs[0], 1), Wrow(hs[1], 1),
        Wrow(hs[2], 0), Wrow(hs[3], 0), Wrow(hs[2], 1), Wrow(hs[3], 1)],
        axis=0)
    bqk_flat = np.concatenate([
        brow(hs[0], 0), brow(hs[1], 0), brow(hs[0], 1), brow(hs[1], 1),
        brow(hs[2], 0), brow(hs[3], 0), brow(hs[2], 1), brow(hs[3], 1)],
        axis=0)
    import ml_dtypes

    bf = ml_dtypes.bfloat16
    WqkT = np.ascontiguousarray(qk_rows.T.astype(bf))        # [1024, 512]
    bqk = np.ascontiguousarray(bqk_flat.reshape(4, P).T)     # [128, 4]
    WvT = np.ascontiguousarray(
        np.concatenate([Wrow(h, 2) for h in hs], axis=0).T.astype(bf))
    dsl = np.concatenate([np.arange(h * 64, (h + 1) * 64) for h in hs])
    WoutT = np.ascontiguousarray(W_out[:, dsl].T.astype(bf))  # [256, 1024]
    yT = np.ascontiguousarray(y[b].T.astype(bf))             # [1024, 2048]
    return {"yT": yT, "WqkT": WqkT, "bqk": bqk, "WvT": WvT,
            "WoutT": WoutT}


def _gather(results, b_qkv, W_out, b_out):
    parts = [results[c]["out"] for c in range(8)]
    # v-bias commutes through the output projection: fold it host-side
    bv_full = b_qkv.reshape(16, 3, 64)[:, 2, :].reshape(1024)
    bias = b_out + bv_full @ W_out.T
    return np.stack([
        parts[0] + parts[1] + parts[2] + parts[3] + bias,
        parts[4] + parts[5] + parts[6] + parts[7] + bias,
    ]).astype(np.float32)


def kernel(y, W_qkv, b_qkv, W_out, b_out):
    from concourse.bass_utils import run_bass_kernel_spmd

    y = np.ascontiguousarray(np.asarray(y, dtype=np.float32))
    W_qkv = np.ascontiguousarray(np.asarray(W_qkv, dtype=np.float32))
    b_qkv = np.ascontiguousarray(np.asarray(b_qkv, dtype=np.float32))
    W_out = np.ascontiguousarray(np.asarray(W_out, dtype=np.float32))
    b_out = np.asarray(b_out, dtype=np.float32)

    nc = _get_nc()
    in_maps = [_host_prep(y, W_qkv, b_qkv, W_out, c) for c in range(8)]
    res = run_bass_kernel_spmd(nc, in_maps, core_ids=list(range(8)))
    return _gather(res.results, b_qkv, W_out, b_out)
